# revision 41
# baseline (speedup 1.0000x reference)
"""Multi-head attention (16 heads, L=2312, E=1024) on 8 trn2 NeuronCores.

Sharding: tensor-parallel over heads — each core computes 2 heads' full
attention (QKV proj + RoPE + softmax(QK^T)V). Phase B is head-major: head 0's
context re-shards (head-split AllToAll #1) while head 1's attention computes;
AllToAll #2 overlaps the first (head-0) half of the k-split output projection.
Host concatenates the per-core row shards.

Numerics: bf16 operands with fp32 PSUM accumulation + fp32 softmax
(exp / sum / reciprocal in fp32).

Self-contained: all shapes hardcoded; takes full unsharded inputs.
"""
import numpy as np
import ml_dtypes

import concourse.bacc as bacc
import concourse.tile as tile
from concourse import mybir
from concourse.bass_utils import run_bass_kernel_spmd
from concourse.masks import make_identity

N_CORES = 8
L = 2312           # valid sequence length
LP = 2432          # padded to 19*128
NK = LP // 128     # 19 key tiles
E = 1024
KE = E // 128      # 8 contraction tiles over embed dim
SHARD = LP // N_CORES  # 304 rows of output per core
F32 = mybir.dt.float32
BF16 = mybir.dt.bfloat16
SCALE = 0.125      # 1/sqrt(64)

# lq blocks: (start, width); widths multiples of 128 except last (2312-2048=264)
LQB = [(0, 512), (512, 512), (1024, 512), (1536, 512), (2048, 264)]
# qkv N blocks over the valid seq (pad cols 2312:2432 of K/V are memset to 0)
NBLK = [(0, 256), (256, 256), (512, 512), (1024, 512), (1536, 512), (2048, 264)]

_NC_CACHE = {}


def _build():
    if "nc" in _NC_CACHE:
        return _NC_CACHE["nc"]
    nc = bacc.Bacc(
        "TRN2",
        target_bir_lowering=False,
        debug=False,
        enable_asserts=False,
        num_devices=N_CORES,
    )
    # x and w arrive chunk-major (host pre-transposed) so every phase-A DMA
    # moves fully contiguous rows
    XCOLS = KE * L  # 18496
    xT = nc.dram_tensor("xT", [128, XCOLS], BF16, kind="ExternalInput").ap()
    wT = nc.dram_tensor("wT", [128, 3 * KE * 128], BF16, kind="ExternalInput").ap()
    bqkv = nc.dram_tensor("bqkv", [128, 3], F32, kind="ExternalInput").ap()
    cosT = nc.dram_tensor("cosT", [128, LP], BF16, kind="ExternalInput").ap()
    sinT = nc.dram_tensor("sinT", [128, LP], BF16, kind="ExternalInput").ap()
    mskT = nc.dram_tensor("mskT", [128, NK], F32, kind="ExternalInput").ap()
    pwT = nc.dram_tensor("pwT", [E, E], BF16, kind="ExternalInput").ap()
    pb = nc.dram_tensor("pb", [128, KE], F32, kind="ExternalInput").ap()
    perm = nc.dram_tensor("perm", [128, 128], BF16, kind="ExternalInput").ap()
    outT = nc.dram_tensor("outT", [E, SHARD], F32, kind="ExternalOutput").ap()

    with tile.TileContext(nc) as tc:
        with (
            tc.tile_pool(name="const", bufs=1) as cpool,
            tc.tile_pool(name="dram", bufs=1, space="DRAM") as dpool,
            tc.tile_pool(name="qkv", bufs=1) as qkvpool,
            tc.tile_pool(name="vaugp", bufs=1) as vaugpool,
            tc.tile_pool(name="ctxp", bufs=1) as ctxpool,
            tc.tile_pool(name="psb", bufs=4) as pspool,
            tc.tile_pool(name="ct", bufs=2) as ctpool,
            tc.tile_pool(name="cn", bufs=2) as cnpool,
            tc.tile_pool(name="rp", bufs=2) as rpool,
            tc.tile_pool(name="pw_ag", bufs=1) as pwpool,
        ):
            ident = cpool.tile([128, 128], F32)
            identb = cpool.tile([128, 128], BF16)
            pbias = cpool.tile([128, KE], F32)
            mask_sb = cpool.tile([128, NK], F32)
            perm_sb = cpool.tile([128, 128], BF16)

            Q = qkvpool.tile([128, LP], BF16)
            Qz0 = qkvpool.tile([128, LP], BF16)  # [rope(q_h0) ; 0]
            Qz1 = qkvpool.tile([128, LP], BF16)  # [0 ; rope(q_h1)]
            K = qkvpool.tile([128, LP], BF16)
            V = qkvpool.tile([128, LP], BF16)
            vaug = vaugpool.tile([128, NK, 130], BF16)
            ctxTn = ctxpool.tile([128, LP], BF16)
            cc_in = [dpool.tile([N_CORES, 64, SHARD], BF16, name=f"cci{h}")
                     for h in range(2)]
            cc_out = [dpool.tile([N_CORES, 64, SHARD], BF16, name=f"cco{h}")
                      for h in range(2)]
            QZ = [Qz0, Qz1]

            # PSUM pool for the context accumulators — spans phases A and B,
            # explicitly closed before phase C needs all 8 banks
            psc_cm = tc.tile_pool(name="ps_c", bufs=2, space="PSUM")
            psc = psc_cm.__enter__()

            # ---------------- Phase A: QKV projection + RoPE + V transpose ----
            with (
                tc.tile_pool(name="xw", bufs=1) as xwpool,
                tc.tile_pool(name="ropet", bufs=3) as rtp,
                tc.tile_pool(name="ps_a", bufs=2, space="PSUM") as psa,
                tc.tile_pool(name="ps_sw", bufs=1, space="PSUM") as psw,
                tc.tile_pool(name="ps_vt", bufs=1, space="PSUM") as psvt,
                tc.tile_pool(name="ps_s0", bufs=1, space="PSUM") as spA,
            ):
                x_sb = xwpool.tile([128, XCOLS], BF16)
                w_sb = xwpool.tile([128, 3 * KE * 128], BF16)
                b_sb = xwpool.tile([128, 3], F32)
                cos_sb = xwpool.tile([128, LP], BF16)
                sin_sb = xwpool.tile([128, LP], BF16)
                # chunk-major column offsets into x_sb per NBLK block
                xoff = {}
                off = 0
                for (n0, nw) in NBLK:
                    xoff[n0] = off
                    off += KE * nw

                # sync queue: first x chunk, then the three w sections, then
                # the remaining x chunks — all fully contiguous
                o0 = xoff[NBLK[0][0]]
                nc.sync.dma_start(x_sb[:, :KE * NBLK[0][1]], xT[:, :KE * NBLK[0][1]])
                for sec in range(3):
                    nc.sync.dma_start(
                        w_sb[:, 1024 * sec:1024 * (sec + 1)],
                        wT[:, 1024 * sec:1024 * (sec + 1)],
                    )
                for (n0, nw) in NBLK[1:]:
                    o = xoff[n0]
                    nc.sync.dma_start(
                        x_sb[:, o:o + KE * nw], xT[:, o:o + KE * nw]
                    )
                # scalar queue is idle until the first exp: rope tables
                nc.scalar.dma_start(cos_sb[:, 0:512], cosT[:, 0:512])
                nc.scalar.dma_start(sin_sb[:, 0:512], sinT[:, 0:512])
                nc.scalar.dma_start(cos_sb[:, 512:1536], cosT[:, 512:1536])
                nc.scalar.dma_start(sin_sb[:, 512:1536], sinT[:, 512:1536])
                nc.scalar.dma_start(cos_sb[:, 1536:L], cosT[:, 1536:L])
                nc.scalar.dma_start(sin_sb[:, 1536:L], sinT[:, 1536:L])
                # gpsimd queue: the small constants
                nc.gpsimd.dma_start(perm_sb[:], perm)
                nc.gpsimd.dma_start(b_sb[:], bqkv)
                nc.gpsimd.dma_start(mask_sb[:], mskT)
                nc.gpsimd.dma_start(pbias[:], pb)

                # identity matrices are built on engines, emitted after the
                # DMA descgens so they don't delay the input transfers
                make_identity(nc, identb[:])
                make_identity(nc, ident[:])

                # static zero regions: pad cols of K/V (masked keys must be
                # finite) and the dead head-half of each zero-padded Q variant
                nc.vector.memset(K[:, L:LP], 0.0)
                nc.vector.memset(V[:, L:LP], 0.0)
                nc.any.memset(Qz0[64:128, :], 0.0)
                nc.any.memset(Qz1[0:64, :], 0.0)
                nc.vector.memset(ctxTn[:, L:LP], 0.0)
                # mask columns of v_aug depend only on the mask DMA
                mview = mask_sb[:].rearrange("p (t o) -> p t o", o=1)
                nc.vector.tensor_copy(vaug[:, :, 64:65], mview)
                nc.vector.tensor_copy(vaug[:, :, 129:130], mview)

                def rope_chunk(T, n0, nw):
                    # rotate T[:, n0:n0+nw]; Q writes into the zero-padded
                    # per-head variants, K rotates in place. The 32-half swap
                    # within each head is a permutation matmul on PE.
                    swp = psw.tile([128, 512], F32, tag="swp", name=f"swp_{T.name}_{n0}")
                    nc.tensor.matmul(swp[:, :nw], perm_sb[:], T[:, n0:n0 + nw])
                    sw = rtp.tile([128, 512], BF16, tag="swap", name=f"sw_{T.name}_{n0}")
                    tmp = rtp.tile([128, 512], BF16, tag="tmp", name=f"tmp_{T.name}_{n0}")
                    nc.vector.tensor_mul(tmp[:, :nw], T[:, n0:n0 + nw], cos_sb[:, n0:n0 + nw])
                    nc.vector.tensor_mul(sw[:, :nw], swp[:, :nw], sin_sb[:, n0:n0 + nw])
                    if T is Q:
                        nc.vector.tensor_add(
                            Qz0[0:64, n0:n0 + nw], tmp[0:64, :nw], sw[0:64, :nw]
                        )
                        nc.vector.tensor_add(
                            Qz1[64:128, n0:n0 + nw], tmp[64:128, :nw], sw[64:128, :nw]
                        )
                    else:
                        nc.vector.tensor_add(T[:, n0:n0 + nw], tmp[:, :nw], sw[:, :nw])

                def vaug_chunk(n0, nw):
                    hi = min(n0 + nw, L)
                    for t in range(n0 // 128, (hi + 127) // 128):
                        tp = psvt.tile([128, 128], BF16, tag="vtp")
                        nc.tensor.transpose(tp[:], V[:, 128 * t:128 * (t + 1)], identb[:])
                        nc.vector.tensor_scalar_mul(
                            vaug[:, t, 0:64], tp[:, 0:64], mask_sb[:, t:t + 1]
                        )
                        nc.vector.tensor_scalar_mul(
                            vaug[:, t, 65:129], tp[:, 64:128], mask_sb[:, t:t + 1]
                        )

                # h0 attention for lq blocks 0 AND 1 is interleaved into phase
                # A, stepped between QKV sections so the in-order PE queue
                # never blocks on the ACT engine. Block 0 starts as soon as
                # 512 query cols are roped (single-block items for tiles 0-3);
                # once 1024 cols are roped, block 1 backfills and the rest run
                # as dual-block items (one 1024-col exp per key tile).
                PC_A = [None, None]  # lazily allocated h0 accumulators (b0, b1)
                att0_items = []      # (mode, t): mode 0 = b0, 1 = b1, 2 = dual
                att0_pend = None     # (mode, t, PSb) awaiting ctx emission

                def att0_ctx(pend):
                    mode, t, pb_ = pend
                    for blk in ([0] if mode == 0 else [1] if mode == 1 else [0, 1]):
                        if PC_A[blk] is None:
                            PC_A[blk] = psc.tile(
                                [128, 512], F32, tag="pc", name=f"pcA{blk}"
                            )
                        nc.tensor.matmul(
                            PC_A[blk][0:65, :],
                            vaug[:, t, 0:65],
                            pb_[:, 512 * (blk if mode == 2 else 0):
                                512 * (blk if mode == 2 else 0) + 512],
                            start=(t == 0),
                            stop=(t == NK - 1),
                        )

                def att0_step():
                    nonlocal att0_pend
                    if not att0_items:
                        return
                    mode, t = att0_items.pop(0)
                    SP = spA.tile([128, 1024], F32, tag="sp0", name="sp0")
                    PSb = pspool.tile([128, 1536], BF16, tag="psb", name="psb")
                    blks = [0] if mode == 0 else [1] if mode == 1 else [0, 1]
                    for i, blk in enumerate(blks):
                        nc.tensor.matmul(
                            SP[:, 512 * i:512 * i + 512],
                            K[:, 128 * t:128 * (t + 1)],
                            Qz0[:, 512 * blk:512 * blk + 512],
                        )
                    w = 512 * len(blks)
                    nc.scalar.activation(
                        PSb[:, :w], SP[:, :w],
                        mybir.ActivationFunctionType.Exp, scale=SCALE,
                    )
                    if att0_pend is not None:
                        att0_ctx(att0_pend)
                    att0_pend = (mode, t, PSb)

                outs = [Q, K, V]
                _att0_hi = [0]
                for (n0, nw) in NBLK:
                    for m in range(3):
                        ps = psa.tile([128, 512], F32, tag="qkvps")
                        for k in range(KE):
                            nc.tensor.matmul(
                                ps[:, :nw],
                                w_sb[:, 1024 * m + 128 * k:1024 * m + 128 * k + 128],
                                x_sb[:, xoff[n0] + nw * k:xoff[n0] + nw * k + nw],
                                start=(k == 0),
                                stop=(k == KE - 1),
                            )
                        nc.vector.tensor_scalar_add(
                            outs[m][:, n0:n0 + nw], ps[:, :nw], b_sb[:, m:m + 1]
                        )
                        if m < 2:
                            rope_chunk(outs[m], n0, nw)
                        else:
                            vaug_chunk(n0, nw)
                        att0_step()
                        att0_step()
                    cov = n0 + nw
                    if cov == 512:
                        # Qz0[:, 0:512] roped; K/vaug tiles 0-3 available
                        att0_items.extend((0, t) for t in range(4))
                        _att0_hi[0] = 4
                    elif cov >= 1024:
                        if _att0_hi[0] == 4:
                            att0_items.extend((1, t) for t in range(4))
                        hi = min(cov, L)
                        att0_items.extend(
                            (2, t) for t in range(max(_att0_hi[0], 4), (hi + 127) // 128)
                        )
                        _att0_hi[0] = (hi + 127) // 128
                while att0_items:
                    att0_step()
                if att0_pend is not None:
                    att0_ctx(att0_pend)
                    att0_pend = None

            # ---------------- Phase B: head-major attention -------------------
            if True:
                # proj weights + re-shard landing buffers load during phase B
                # on the now-idle sync DMA queue
                pw_sb = pwpool.tile([128, KE, E], BF16)
                pwr = pwT.rearrange("(k p) e -> p k e", p=128)
                nc.sync.dma_start(pw_sb[:, 0:4, :], pwr[:, 0:4, :])
                nc.sync.dma_start(pw_sb[:, 4:8, :], pwr[:, 4:8, :])
                ag = [pwpool.tile([128, 4, SHARD], BF16, name=f"ag{h}")
                      for h in range(2)]
                osb = pwpool.tile([128, KE, SHARD], F32)

                with (
                    tc.tile_pool(name="ps_s", bufs=2, space="PSUM") as pss,
                ):
                    norm_q = []      # deferred norms: dicts of subs/cb/atomic
                    cur_norm = [None]
                    cc_next = [0, 0]
                    pend = []        # ctx groups trailing the score stream

                    def norm_step():
                        # run one deferred-norm subtile (or a whole atomic
                        # entry) — called at score-group boundaries so the
                        # divide dance never stalls the exp stream
                        while norm_q and not cur_norm[0] and norm_q[0]["atomic"]:
                            e = norm_q.pop(0)
                            for s_fn in e["subs"]:
                                s_fn()
                            if e["cb"]:
                                e["cb"]()
                        if not cur_norm[0] and norm_q:
                            cur_norm[0] = norm_q.pop(0)
                        e = cur_norm[0]
                        if e:
                            e["subs"].pop(0)()
                            if not e["subs"]:
                                if e["cb"]:
                                    e["cb"]()
                                cur_norm[0] = None

                    def norm_subs(h, lq0, lqw, bst):
                        # softmax divide: transpose so queries become the
                        # partition dim, scale by 1/denominator, transpose
                        # back — in bf16 (1 PE cycle/row instead of 2; the
                        # context is bf16 downstream anyway). Returns one
                        # closure per 128-query subtile so the PE work can be
                        # spread across score-group boundaries. TT shares the
                        # psc slot rotation with the PC accumulators.
                        state = {}
                        nsub = (lqw + 127) // 128

                        def sub(s):
                            if s == 0:
                                CT = ctpool.tile([65, 512], BF16, tag="ct", name="ct")
                                nc.vector.tensor_copy(
                                    CT[:, :lqw], bst["PC"][0:65, :lqw]
                                )
                                state["CT"] = CT
                                state["TTb"] = psc.tile(
                                    [128, 512], F32, tag="pc", name="tt"
                                )[:].bitcast(BF16)
                            CT, TTb = state["CT"], state["TTb"]
                            sw_ = min(128, lqw - 128 * s)
                            nc.tensor.transpose(
                                TTb[:sw_, 256 * (s % 2):256 * (s % 2) + 65],
                                CT[:, 128 * s:128 * s + sw_],
                                identb[:65, :65],
                            )
                            Rc = rpool.tile([128, 1], F32, tag="rc", name="rc")
                            nc.vector.reciprocal(
                                Rc[:sw_, :], TTb[:sw_, 256 * (s % 2) + 64:256 * (s % 2) + 65]
                            )
                            CN = cnpool.tile([128, 64], BF16, tag="cn", name="cn")
                            nc.vector.tensor_scalar_mul(
                                CN[:sw_, :], TTb[:sw_, 256 * (s % 2):256 * (s % 2) + 64],
                                Rc[:sw_, :],
                            )
                            nc.tensor.transpose(
                                TTb[0:64, 256 * (s % 2) + 128:256 * (s % 2) + 128 + sw_],
                                CN[:sw_, :], identb[:sw_, :sw_]
                            )
                            nc.vector.tensor_copy(
                                ctxTn[64 * h:64 * h + 64,
                                      lq0 + 128 * s:lq0 + 128 * s + sw_],
                                TTb[0:64, 256 * (s % 2) + 128:256 * (s % 2) + 128 + sw_],
                            )
                            # ship any query shard this subtile completed
                            # (gpsimd descgen queue — off the critical path)
                            done = lq0 + 128 * s + sw_
                            if s == nsub - 1 and lqw != 512:
                                done = LP  # pad cols of ctxTn are zeroed
                            while cc_next[h] * SHARD + SHARD <= done:
                                j = cc_next[h]
                                nc.gpsimd.dma_start(
                                    cc_in[h][j],
                                    ctxTn[64 * h:64 * h + 64,
                                          SHARD * j:SHARD * (j + 1)],
                                )
                                cc_next[h] += 1

                        return [(lambda s=s: sub(s)) for s in range(nsub)]

                    def flush_pend(n_keep):
                        while len(pend) > n_keep:
                            tl_p, pb_, bst = pend.pop(0)
                            if bst["PC"] is None:
                                bst["PC"] = psc.tile(
                                    [128, 512], F32, tag="pc",
                                    name=f"pc{bst['h']}_{bst['lq0']}",
                                )
                            for i, t in enumerate(tl_p):
                                nc.tensor.matmul(
                                    bst["PC"][0:65, :bst["lqw"]],
                                    vaug[:, t, 65 * bst["h"]:65 * bst["h"] + 65],
                                    pb_[:, 512 * i:512 * i + bst["lqw"]],
                                    start=(t == 0),
                                    stop=(t == NK - 1),
                                )

                    def attention_pass(h, blocks):
                        for (lq0, lqw) in blocks:
                            bst = {"PC": None, "h": h, "lq0": lq0, "lqw": lqw}
                            groups = [list(range(g, min(g + 3, NK)))
                                      for g in range(0, NK, 3)]
                            for gi, tl in enumerate(groups):
                                SP = pss.tile([128, 1536], F32, tag="sp", name="sp")
                                for i, t in enumerate(tl):
                                    nc.tensor.matmul(
                                        SP[:, 512 * i:512 * i + lqw],
                                        K[:, 128 * t:128 * (t + 1)],
                                        QZ[h][:, lq0:lq0 + lqw],
                                    )
                                PSb = pspool.tile([128, 1536], BF16, tag="psb", name="psb")
                                n = len(tl)
                                if lqw == 512:
                                    src = SP[:, :512 * n]
                                    dst = PSb[:, :512 * n]
                                else:
                                    src = SP[:].rearrange(
                                        "p (g q) -> p g q", q=512)[:, 0:n, 0:lqw]
                                    dst = PSb[:].rearrange(
                                        "p (g q) -> p g q", q=512)[:, 0:n, 0:lqw]
                                nc.scalar.activation(
                                    dst, src, mybir.ActivationFunctionType.Exp,
                                    scale=SCALE,
                                )
                                # by gi==2 the previous block's last ctx group
                                # has flushed — drip its deferred norm
                                # subtiles while this block's exp stream keeps
                                # ACT busy
                                if gi >= 2:
                                    norm_step()
                                pend.append((tl, PSb, bst))
                                flush_pend(2)
                            norm_q.append({
                                "subs": norm_subs(h, lq0, lqw, bst),
                                "cb": None,
                                "atomic": False,
                            })

                    def launch_cc0():
                        # h0 re-shard fires as soon as its last shard is
                        # copied — overlapped with the rest of the h1 pass
                        nc.gpsimd.collective_compute(
                            "AllToAll",
                            mybir.AluOpType.bypass,
                            replica_groups=[list(range(N_CORES))],
                            ins=[cc_in[0].opt()],
                            outs=[cc_out[0].opt()],
                        )
                        ccr0 = cc_out[0][:].rearrange("(k i) d n -> (i d) k n", i=2)
                        # only queues idle during the h1 pass — a chunk on the
                        # scalar/vector queues would block the exp stream
                        # behind the AllToAll wait
                        for k, eng in enumerate((nc.gpsimd, nc.sync, nc.gpsimd, nc.sync)):
                            eng.dma_start(ag[0][:, k, :], ccr0[:, k, :])

                    # blocks 0-1 of head 0 came from phase A: queue their
                    # norms (atomic, so the psc slot rotation stays paired)
                    b0, b1 = LQB[0], LQB[1]
                    norm_q.append({
                        "subs": norm_subs(0, b0[0], b0[1], {"PC": PC_A[0]}),
                        "cb": None, "atomic": True,
                    })
                    norm_q.append({
                        "subs": norm_subs(0, b1[0], b1[1], {"PC": PC_A[1]}),
                        "cb": None, "atomic": False,
                    })
                    # head 0 blocks 2-4, then head 1 all blocks
                    attention_pass(0, LQB[2:])
                    norm_q[-1]["cb"] = launch_cc0
                    attention_pass(1, LQB)
                    # drain: last ctx groups, h1's final norm + shard copies
                    flush_pend(0)
                    while norm_q or cur_norm[0]:
                        norm_step()
                    nc.gpsimd.collective_compute(
                        "AllToAll",
                        mybir.AluOpType.bypass,
                        replica_groups=[list(range(N_CORES))],
                        ins=[cc_in[1].opt()],
                        outs=[cc_out[1].opt()],
                    )
                    ccr1 = cc_out[1][:].rearrange("(k i) d n -> (i d) k n", i=2)
                    for k, eng in enumerate((nc.gpsimd, nc.sync, nc.scalar, nc.gpsimd)):
                        eng.dma_start(ag[1][:, k, :], ccr1[:, k, :])

                # ------------ Phase C: output projection ----------------------
                # k 0..3 (head-0 dims) depend only on AllToAll #1 — the PE
                # churns through them while AllToAll #2 is still in flight
                psc_cm.__exit__(None, None, None)
                outTr = outT.rearrange("(k p) n -> p k n", p=128)
                with tc.tile_pool(name="ps_o", bufs=1, space="PSUM") as pso:
                    pos = [
                        pso.tile([128, SHARD], F32, tag=f"po{mE}", name=f"po{mE}")
                        for mE in range(KE)
                    ]
                    # head-0 half: runs on the PE while AllToAll #2 is still
                    # in flight (depends only on ag[0])
                    for k in range(4):
                        for mE in range(KE):
                            nc.tensor.matmul(
                                pos[mE][:],
                                pw_sb[:, k, 128 * mE:128 * (mE + 1)],
                                ag[0][:, k, :],
                                start=(k == 0),
                                stop=False,
                            )
                    # head-1 half: mE-major so each output chunk's bias-add
                    # and store overlap the remaining chunks' matmuls
                    for mE in range(KE):
                        for k in range(4):
                            nc.tensor.matmul(
                                pos[mE][:],
                                pw_sb[:, 4 + k, 128 * mE:128 * (mE + 1)],
                                ag[1][:, k, :],
                                start=False,
                                stop=(k == 3),
                            )
                        nc.vector.tensor_scalar_add(
                            osb[:, mE, :], pos[mE][:], pbias[:, mE:mE + 1]
                        )
                        eng = nc.sync if mE % 2 == 0 else nc.gpsimd
                        eng.dma_start(outTr[:, mE, :], osb[:, mE, :])

    nc.compile()
    _NC_CACHE["nc"] = nc
    return nc


def _prep_inputs(x, key_padding_mask, qkv_w, qkv_b, proj_w, proj_b, freqs_cos, freqs_sin):
    bf = ml_dtypes.bfloat16
    x = np.ascontiguousarray(np.asarray(x, np.float32))
    qkv_w = np.asarray(qkv_w, np.float32)
    qkv_b = np.asarray(qkv_b, np.float32)
    proj_w = np.asarray(proj_w, np.float32)
    proj_b = np.asarray(proj_b, np.float32)
    fc = np.asarray(freqs_cos, np.float32)  # [2304, 64]
    fs = np.asarray(freqs_sin, np.float32)
    mask = np.asarray(key_padding_mask)

    # chunk-major x: per NBLK block a contiguous [128, KE*nw] slab with
    # column order (k, n) — matches the kernel's x_sb layout
    xTf = x.T.astype(bf)  # [E, L]
    xH = np.concatenate(
        [
            xTf[:, n0:n0 + nw].reshape(KE, 128, nw).transpose(1, 0, 2).reshape(128, KE * nw)
            for (n0, nw) in NBLK
        ],
        axis=1,
    )
    xH = np.ascontiguousarray(xH)

    cosT = np.ones((64, LP), np.float32)
    cosT[:, 8:L] = fc.T
    cos2 = np.concatenate([cosT, cosT], axis=0).astype(bf)  # [128, LP]

    sinT = np.zeros((64, LP), np.float32)
    sinT[:, 8:L] = fs.T
    sinT[:32, :] *= -1.0  # sign of -x2 half folded into sin table
    sin2 = np.concatenate([sinT, sinT], axis=0).astype(bf)

    maskf = np.zeros((LP,), np.float32)
    maskf[:L] = mask.astype(np.float32)
    mskT = np.ascontiguousarray(maskf.reshape(NK, 128).T)  # [128, NK]

    # proj_w rows permuted to the head-split AllToAll arrival order:
    # [all cores' head-0 dims, all cores' head-1 dims]
    pidx = np.concatenate([
        np.concatenate([np.arange(64) + (2 * c + hh) * 64 for c in range(N_CORES)])
        for hh in range(2)
    ])
    pwT = np.ascontiguousarray(proj_w.T[pidx]).astype(bf)  # [d, e]
    permM = np.zeros((128, 128), np.float32)  # lhsT: permM[k, m]=1 iff k==swap(m)
    for m128 in range(128):
        swp = m128 + 32 if (m128 % 64) < 32 else m128 - 32
        permM[swp, m128] = 1.0
    permM = permM.astype(bf)
    pb2 = np.ascontiguousarray(proj_b.reshape(KE, 128).T)  # [128, KE]

    in_maps = []
    for c in range(N_CORES):
        h0, h1 = 2 * c, 2 * c + 1
        rows = []
        bias_rows = []
        for sec in range(3):  # q, k, v sections of qkv_w
            for h in (h0, h1):
                sl = slice(1024 * sec + 64 * h, 1024 * sec + 64 * h + 64)
                rows.append(qkv_w[sl])
                bias_rows.append(qkv_b[sl])
        Wc = np.concatenate(rows, axis=0)           # [384, 1024]
        bc = np.concatenate(bias_rows, axis=0)      # [384]
        # section-major w: columns ordered (sec, k, c) so each q/k/v section
        # is one contiguous [128, 1024] DMA
        WcT = Wc.T.astype(bf)  # [1024, 384]
        wH = np.ascontiguousarray(
            WcT.reshape(KE, 128, 3, 128).transpose(1, 2, 0, 3).reshape(128, 3 * KE * 128)
        )
        in_maps.append({
            "xT": xH,
            "wT": wH,
            "bqkv": np.ascontiguousarray(bc.reshape(3, 128).T),
            "cosT": cos2,
            "sinT": sin2,
            "mskT": mskT,
            "pwT": pwT,
            "pb": pb2,
            "perm": permM,
        })
    return in_maps


def _run(in_maps, trace=False):
    nc = _build()
    return run_bass_kernel_spmd(
        nc, in_maps, core_ids=list(range(N_CORES)), trace=trace
    )


def kernel(x, key_padding_mask, qkv_w, qkv_b, proj_w, proj_b, freqs_cos, freqs_sin):
    in_maps = _prep_inputs(
        x, key_padding_mask, qkv_w, qkv_b, proj_w, proj_b, freqs_cos, freqs_sin
    )
    res = _run(in_maps, trace=False)
    outT_full = np.concatenate(
        [res.results[c]["outT"] for c in range(N_CORES)], axis=1
    )  # [E, LP]
    return np.ascontiguousarray(outT_full[:, :L].T).astype(np.float32)
